# revision 28
# baseline (speedup 1.0000x reference)
"""Trainium2 Bass kernel for nn_DoublyStochasticButterfly (v4).

Math
----
reference applies 20 butterfly stages; conjugating the fixed perfect shuffle
away, stage t mixes feature pairs (i0, i1 = i0 | 1<<b) differing in bit
b = (9-t) % 10, with weight params[p, t] at i0 = rotr^t(p).  shuffle^20 =
identity, so no output permutation is needed.  The network splits as

    stage 0 (bit 9)  C1A  elementwise butterfly   (DVE custom lerp)
    stage 1 (bit 8)  C1B  elementwise butterfly   (DVE custom lerp)
    stages 2..10     G1*  composed; block-diagonal under fixed bit 8 ->
                     2 interleaved 512-blocks over tiles {0,1,4,5} / {2,3,6,7}
                     (TensorE bf16, full-chunk 512-wide matmuls)
    stages 11..19    G2*  composed; contiguous 512-blocks (TensorE bf16,
                     swapped operands -> batch-major f32 PSUM output)

Layout: input is transposed + cast to bf16 on the host (feature-major), so
no PE transposes are needed and input DMA is halved; both butterflies run at
the front on SBUF.  G1* computes one 512-block (4 feature tiles x 512 rows)
into a 4-bank PSUM tile at a time; ScalarE copies it to SBUF (bf16) for
G2*'s lhsT; G2* writes batch-major f32 into the other PSUM banks, ScalarE
evacuates and SyncE DMAs out.  PE order g1A(c), g1B(c), g2(c-1,1), g2(c,0)
keeps every ScalarE copy one full PE phase ahead of its consumer.

Sharding: batch dim split across the 8 cores (data parallel, no comm).
"""

import numpy as np

# ---------------------------------------------------------------- constants
WIDTH = 1024
HALF = 512
DEPTH = 20
BATCH = 32768
NCORES = 8
BSH = BATCH // NCORES  # 4096 rows per core
CHUNK = 512  # batch rows per pipeline chunk
NCHUNK = BSH // CHUNK
NT = 8  # feature tiles of 128

REPEAT = 1  # device-side repeat of the whole pipeline (for benchmarking)

# front butterfly schedule: (stage, low tiles, partner stride)
_FRONT = [
    (0, (0, 1, 2, 3), 4),   # bit 9
    (1, (0, 1, 4, 5), 2),   # bit 8
    (2, (0, 2, 4, 6), 1),   # bit 7 (only used by deep-front chunks)
]
_BLOCKS = ((0, 1, 4, 5), (2, 3, 6, 7))  # G1* blocks (bit 8 = 0 / 1)
_POS = {jt: _BLOCKS[(jt >> 1) & 1].index(jt) for jt in range(NT)}
# chunks that run 3 front stages + 256-wide G1 contraction (PE<->DVE balance)
_DEEP = (2, 4, 6)


def _rotr(i, t):
    for _ in range(t):
        i = (i >> 1) | ((i & 1) << 9)
    return i


def _host_precompute(params):
    """Butterfly weight vectors + composed block matrices (f64 host math)."""
    p64 = np.asarray(params, dtype=np.float64)

    def stage_pairs(t):
        b = (9 - t) % 10
        i0 = np.array([_rotr(p, t) for p in range(HALF)])
        i1 = i0 | (1 << b)
        return i0, i1

    def stage_matrix(t):
        m = np.zeros((WIDTH, WIDTH))
        i0, i1 = stage_pairs(t)
        w = p64[:, t]
        m[i0, i0] = 1 - w
        m[i0, i1] = w
        m[i1, i0] = w
        m[i1, i1] = 1 - w
        return m

    wc = np.zeros((128, 12), dtype=np.float64)
    for s_idx, (t, m0s, _stride) in enumerate(_FRONT):
        wt = np.zeros(WIDTH)
        i0, _ = stage_pairs(t)
        wt[i0] = p64[:, t]
        for pi, m0 in enumerate(m0s):
            wc[:, 4 * s_idx + pi] = wt[m0 * 128 : (m0 + 1) * 128]

    def composed(ts):
        g = np.eye(WIDTH)
        for t in ts:
            g = stage_matrix(t) @ g
        return g

    g1 = composed(range(2, 11))   # stages 2..10 (bit-8 blocks, C=512)
    g1d = composed(range(3, 11))  # stages 3..10 (bit-{8,7} blocks, C=256)
    g2 = composed(range(11, 20))  # stages 11..19

    r = np.arange(WIDTH)
    mask = (r[:, None] & 256) == (r[None, :] & 256)
    assert np.abs(g1 * ~mask).max() < 1e-12, "G1* not bit-8-block-diagonal"
    maskd = (r[:, None] & 0b110000000) == (r[None, :] & 0b110000000)
    assert np.abs(g1d * ~maskd).max() < 1e-12, "G1d not bit-{8,7}-blocked"
    assert np.abs(g2[:512, 512:]).max() < 1e-12, "G2* not 512-block-diagonal"
    assert np.abs(g2[512:, :512]).max() < 1e-12, "G2* not 512-block-diagonal"

    # WL1 tile order: block bi, out-tile index ki, in-tile index j
    wl1 = np.zeros((128, 32 * 128))
    for bi, blk in enumerate(_BLOCKS):
        for ki, k in enumerate(blk):
            for j, jt in enumerate(blk):
                idx = (bi * 4 + ki) * 4 + j
                wl1[:, 128 * idx : 128 * (idx + 1)] = g1[
                    128 * k : 128 * (k + 1), 128 * jt : 128 * (jt + 1)
                ].T

    # WL1D: deep-front G1 (C=256): out tile k contracts {k&3, (k&3)|4}
    wl1d = np.zeros((128, 16 * 128))
    for bi, blk in enumerate(_BLOCKS):
        for ki, k in enumerate(blk):
            for j in range(2):
                jt = (k & 3) | (4 * j)
                idx = (bi * 4 + ki) * 2 + j
                wl1d[:, 128 * idx : 128 * (idx + 1)] = g1d[
                    128 * k : 128 * (k + 1), 128 * jt : 128 * (jt + 1)
                ].T

    # WR2[:, 512*jt:...] = G2* block column-slice transposed (operand swap)
    wr2 = np.zeros((128, 8 * 512))
    for q2 in range(2):
        b2 = g2[512 * q2 : 512 * (q2 + 1), 512 * q2 : 512 * (q2 + 1)]
        for j in range(4):
            wr2[:, 512 * (4 * q2 + j) : 512 * (4 * q2 + j + 1)] = (
                b2[:, 128 * j : 128 * (j + 1)].T
            )

    import ml_dtypes

    return (
        wc.astype(np.float32),
        wl1.astype(ml_dtypes.bfloat16),
        wl1d.astype(ml_dtypes.bfloat16),
        wr2.astype(ml_dtypes.bfloat16),
    )


# ---------------------------------------------------------------- custom op
_LERP = None


def _register_lerp():
    """out = (in0 - in1)*s0 + in1, s0 per-partition.  One DVE op per
    butterfly output: a = lerp(x1, x0, w), b = lerp(x0, x1, w)."""
    global _LERP
    if _LERP is not None:
        return _LERP
    from concourse import dve_ops as D
    from concourse.dve_spec import C0, Spec, Src0, Src1, lower
    from concourse.dve_uop import DveOpSpec

    name = "LERP_ANT_BFLY"
    for op in D.OPS:
        if op.name == name:
            _LERP = op
            return op
    spec = Spec(
        body=(Src0 - Src1) * C0 + Src1,
        reference=lambda in0, in1, s0, s1, imm2: (in0 - in1) * s0 + in1,
    )
    opcode = D._CUSTOM_DVE_ROW_BASE + len(D.OPS)
    shas = {}
    for ver in ("v3", "v4"):
        uops = lower(spec, ver=ver)
        shas[ver] = DveOpSpec(name=name, opcode=opcode, uops=uops, rd1_en=True).sha(
            ver
        )
    op = D.DveOp(name, spec, subdim=False, uops_sha=shas)
    D.OPS.append(op)
    D.CUSTOM_DVE_SPECS[name] = spec
    D._SUB_OPCODE_FOR_NAME[name] = opcode
    _LERP = op
    return op


# ---------------------------------------------------------------- bass build
_NC_CACHE = None


def _build_nc():
    global _NC_CACHE
    if _NC_CACHE is not None:
        return _NC_CACHE
    import concourse.mybir as mybir
    import concourse.tile as tile
    from concourse import bacc

    lerp = _register_lerp()
    f32 = mybir.dt.float32
    bf16 = mybir.dt.bfloat16

    nc = bacc.Bacc("TRN2", target_bir_lowering=False, debug=False,
                   num_devices=NCORES)
    xt_d = nc.dram_tensor("XT", [WIDTH, BSH], bf16, kind="ExternalInput").ap()
    wl1_d = nc.dram_tensor("WL1", [128, 4096], bf16, kind="ExternalInput").ap()
    wl1d_d = nc.dram_tensor("WL1D", [128, 2048], bf16, kind="ExternalInput").ap()
    wr2_d = nc.dram_tensor("WR2", [128, 4096], bf16, kind="ExternalInput").ap()
    wc_d = nc.dram_tensor("WC", [128, 12], f32, kind="ExternalInput").ap()
    y_d = nc.dram_tensor("Y", [BSH, WIDTH], f32, kind="ExternalOutput").ap()

    with tile.TileContext(nc) as tc:
        with (
            tc.tile_pool(name="wts", bufs=1) as wpool,
            tc.tile_pool(name="io", bufs=3) as iopool,
            tc.tile_pool(name="work", bufs=2) as wkpool,
            tc.tile_pool(name="ps", bufs=2, space="PSUM") as pspool,
        ):
            wl1 = wpool.tile([128, 4096], bf16, tag="wl1")
            wl1d = wpool.tile([128, 2048], bf16, tag="wl1d")
            wr2 = wpool.tile([128, 4096], bf16, tag="wr2")
            wc = wpool.tile([128, 12], f32, tag="wc")
            nc.sync.dma_start(out=wc[:], in_=wc_d[:])

            xin_of, c1_of, u_of, psb_of, psc_of, yv_of = {}, {}, {}, {}, {}, {}

            def dma_in(c):
                xt_ = iopool.tile([128, NT * CHUNK], bf16, tag="xin", bufs=3,
                                  name=f"xin{c}")
                if c == 0:
                    # chunk 0 in tile-pair pieces ({0,1},{4,5} first) so the
                    # first butterfly pairs (m, m+4) can start early
                    for t0_ in (0, 4, 2, 6):
                        nc.sync.dma_start(
                            out=xt_[:, t0_ * CHUNK : (t0_ + 2) * CHUNK].rearrange(
                                "p (t r) -> p t r", r=CHUNK
                            ),
                            in_=xt_d[
                                128 * t0_ : 128 * (t0_ + 2),
                                c * CHUNK : (c + 1) * CHUNK,
                            ].rearrange("(t p) r -> p t r", p=128),
                        )
                else:
                    nc.sync.dma_start(
                        out=xt_[:].rearrange("p (t r) -> p t r", r=CHUNK),
                        in_=xt_d[:, c * CHUNK : (c + 1) * CHUNK].rearrange(
                            "(t p) r -> p t r", p=128
                        ),
                    )
                xin_of[c] = xt_

            def fronts(c):
                """Butterfly stages 0,1 (+2 for deep-front chunks), all-DVE."""
                deep = c in _DEEP
                xin = xin_of.pop(c)
                t0 = wkpool.tile([128, NT * CHUNK], bf16, tag="t0", bufs=2,
                                 name=f"t0_{c}")
                c1 = wkpool.tile([128, NT * CHUNK], bf16, tag="c1", bufs=2,
                                 name=f"c1_{c}")
                bufs = [xin, t0, c1]
                if deep:
                    t1 = wkpool.tile([128, NT * CHUNK], bf16, tag="t1", bufs=2,
                                     name=f"t1_{c}")
                    bufs = [xin, t0, t1, c1]
                c1_of[c] = c1

                def tl(buf, m):
                    return buf[:, m * CHUNK : (m + 1) * CHUNK]

                def pair(s_idx, src, dst, pi, m0, side):
                    m1 = m0 + _FRONT[s_idx][2]
                    w = wc[:, 4 * s_idx + pi : 4 * s_idx + pi + 1]
                    x0, x1 = tl(src, m0), tl(src, m1)
                    if side == 0:
                        nc.vector._custom_dve(lerp, out=tl(dst, m0), in0=x1,
                                              in1=x0, s0=w)
                    else:
                        nc.vector._custom_dve(lerp, out=tl(dst, m1), in0=x0,
                                              in1=x1, s0=w)

                # stage 0: pair order (0,4),(1,5) then (2,6),(3,7) — matches
                # the chunk-0 input DMA piece order
                for pi in range(4):
                    pair(0, bufs[0], bufs[1], pi, pi, 0)
                    pair(0, bufs[0], bufs[1], pi, pi, 1)
                # stage 1: a-sides write exactly the A-block tiles {0,1,4,5}
                for side in range(2):
                    for pi, m0 in enumerate(_FRONT[1][1]):
                        pair(1, bufs[1], bufs[2], pi, m0, side)
                if deep:
                    # stage 2: pairs (0,1),(4,5) write A-tiles; (2,3),(6,7) B
                    for pi, m0 in ((0, 0), (2, 4), (1, 2), (3, 6)):
                        pair(2, bufs[2], bufs[3], pi, m0, 0)
                        pair(2, bufs[2], bufs[3], pi, m0, 1)

            def g1(c, bi):
                """One G1 block-half: 4 out-tiles x full chunk -> 4 PSUM banks.

                Normal chunks contract 4 tiles (C=512, stages 2..10); deep
                chunks contract 2 (C=256, stages 3..10)."""
                deep = c in _DEEP
                c1 = c1_of[c]
                psb = pspool.tile([128, 4 * CHUNK], f32, tag="ps", bufs=2,
                                  name=f"psb{c}_{bi}")
                blk = _BLOCKS[bi]
                for ki in range(4):
                    k = blk[ki]
                    dst = psb[:, ki * CHUNK : (ki + 1) * CHUNK]
                    nj = 2 if deep else 4
                    for j in range(nj):
                        if deep:
                            jt = (k & 3) | (4 * j)
                            idx = (bi * 4 + ki) * 2 + j
                            w = wl1d[:, 128 * idx : 128 * (idx + 1)]
                        else:
                            jt = blk[j]
                            idx = (bi * 4 + ki) * 4 + j
                            w = wl1[:, 128 * idx : 128 * (idx + 1)]
                        nc.tensor.matmul(
                            dst,
                            w,
                            c1[:, jt * CHUNK : (jt + 1) * CHUNK],
                            start=(j == 0),
                            stop=(j == nj - 1),
                        )
                psb_of[(c, bi)] = psb
                if bi == 1:
                    c1_of.pop(c)

            def ucopy(c, bi):
                psb = psb_of.pop((c, bi))
                u = wkpool.tile([128, 4 * CHUNK], bf16, tag=f"u{bi}", bufs=2,
                                name=f"u{c}_{bi}")
                nc.scalar.copy(u[:], psb[:])
                u_of[(c, bi)] = u

            def g2(c, h):
                """G2*: swapped operands -> batch-major psc [2s x 1024f]."""
                ua = u_of[(c, 0)]
                ub = u_of[(c, 1)]
                psc = pspool.tile([128, 2 * WIDTH], f32, tag="ps", bufs=2,
                                  name=f"psc{c}_{h}")
                for s in range(2):
                    sg = 2 * h + s
                    for q in range(2):
                        dst = psc[:, s * WIDTH + q * 512 : s * WIDTH + (q + 1) * 512]
                        for j in range(4):
                            jt = 4 * q + j
                            u = ua if ((jt >> 1) & 1) == 0 else ub
                            pos = _POS[jt]
                            nc.tensor.matmul(
                                dst,
                                u[:, pos * CHUNK + sg * 128 : pos * CHUNK + (sg + 1) * 128],
                                wr2[:, 512 * jt : 512 * (jt + 1)],
                                start=(j == 0),
                                stop=(j == 3),
                            )
                psc_of[(c, h)] = psc
                if h == 1:
                    u_of.pop((c, 0))
                    u_of.pop((c, 1))

            def evac(c, h):
                # s-granular evac + DMA: shortens the drain at the tail and
                # lets the output DMA start one ScalarE op earlier.  The last
                # chunk's copies run on the (tail-idle) DVE so they overlap
                # ScalarE's remaining psb->u copies.
                psc = psc_of.pop((c, h))
                for s in range(2):
                    yv = iopool.tile([128, WIDTH], f32, tag="yout", bufs=4,
                                     name=f"yv{c}_{h}_{s}")
                    nc.scalar.copy(yv[:], psc[:, s * WIDTH : (s + 1) * WIDTH])
                    yv_of[(c, h, s)] = yv

            def dma_out(c, h):
                r0 = c * CHUNK + h * 256
                for s in range(2):
                    yv = yv_of.pop((c, h, s))
                    nc.sync.dma_start(
                        out=y_d[r0 + s * 128 : r0 + (s + 1) * 128, :],
                        in_=yv[:],
                    )

            # software pipeline: fronts run one chunk ahead of the GEMMs;
            # PE order per chunk: g1A(c), g1B(c), g2(c-1,1), g2(c,0).
            import contextlib

            rep_ctx = (
                tc.For_i(0, REPEAT, 1) if REPEAT > 1 else contextlib.nullcontext()
            )
            # weights on the ACT HWDGE ring so chunk-0's input DMA (SP ring)
            # is not queued behind them
            nc.scalar.dma_start(out=wl1[:], in_=wl1_d[:])
            nc.scalar.dma_start(out=wl1d[:], in_=wl1d_d[:])
            nc.scalar.dma_start(out=wr2[:], in_=wr2_d[:])

            with rep_ctx:
                dma_in(0)
                fronts(0)
                for c in range(NCHUNK):
                    if c + 1 < NCHUNK:
                        dma_in(c + 1)
                    g1(c, 0)
                    ucopy(c, 0)
                    g1(c, 1)
                    ucopy(c, 1)
                    if c + 1 < NCHUNK:
                        fronts(c + 1)
                    if c > 0:
                        g2(c - 1, 1)
                        evac(c - 1, 1)
                        dma_out(c - 1, 1)
                    g2(c, 0)
                    evac(c, 0)
                    dma_out(c, 0)
                g2(NCHUNK - 1, 1)
                evac(NCHUNK - 1, 1)
                dma_out(NCHUNK - 1, 1)

    nc.finalize()
    _NC_CACHE = nc
    return nc


# ---------------------------------------------------------------- entry
def _make_in_maps(X, params):
    import ml_dtypes

    X = np.asarray(X, dtype=np.float32)
    wc, wl1, wl1d, wr2 = _host_precompute(params)
    return [
        {
            "XT": X[c * BSH : (c + 1) * BSH].T.astype(ml_dtypes.bfloat16),
            "WL1": wl1,
            "WL1D": wl1d,
            "WR2": wr2,
            "WC": wc,
        }
        for c in range(NCORES)
    ]


def kernel(X, params):
    nc = _build_nc()
    in_maps = _make_in_maps(X, params)

    from concourse.bass_utils import run_bass_kernel_spmd

    res = run_bass_kernel_spmd(nc, in_maps, core_ids=list(range(NCORES)))
    return np.concatenate([res.results[c]["Y"] for c in range(NCORES)], axis=0)
